# revision 26
# baseline (speedup 1.0000x reference)
"""Trainium2 Bass kernel for nn_MessageArMLP (GNN message passing).

message[e, r, a, c] = node_feat[sender[e], r, a, c]
                      * sigmoid(rc[e] @ W[group(a)])[c] * cutoff[e]

Strategy (PE replication, no per-edge DMA gather):
  * Host sorts edges by sender; core i owns edges whose sender is in
    nodes [1250*i, 1250*(i+1)).  Senders within a core are grouped into
    ten 128-node tiles; each tile's edges fill m_t 128-edge blocks.
  * The core's 1280-node slice of node_feat (bf16) is DMA'd contiguously
    into SBUF once — each node row crosses HBM exactly once (vs. 12x for
    a per-edge gather).
  * Per 128-edge block, a 128x128 0/1 selection matrix S (built on host)
    replicates sender rows via a PE matmul chain (512/448 cols into psA,
    320 into psC — each inside one 2KB PSUM bank):
        psum[e, :] = sum_n S[n, e] * tile[n, :]
  * PSUM evacuation is split across engines: ACT copies cols [0,960) to
    SBUF bf16 and one big DVE 16-bit multiply per chunk applies the
    decay at ~2x rate; DVE multiplies cols [960,1280) straight out of
    PSUM (f32 x bf16 decay).  Pool builds the per-edge decay broadcast
    (sigmoid(rc@W[g]) x cutoff) a chunk ahead of use.  Output is bf16
    (the 2e-2 rel-err budget dwarfs bf16's ~1% worst-case error); the
    host upcasts to f32 and unpermutes to original edge order.
  * HBM traffic/core: ~41.6 MB store + ~8.5 MB loads, no gather reads.
  * Measured: ~246 us vs 496 us for the f32 DMA-gather baseline.
"""

import numpy as np
import ml_dtypes
from contextlib import ExitStack

import concourse.tile as tile
from concourse import bacc, mybir
from concourse.bass_utils import run_bass_kernel_spmd

dt = mybir.dt

# Problem constants (hardcoded per harness contract)
N_NODES = 10000
E_TOTAL = 120000
RADIAL = 8
ANG = 20
CH = 8
REMB = 8
ROW = RADIAL * ANG * CH     # 1280 elems per node row (2560 B bf16)
G = 4
GC = G * CH                 # 32
N_CORES = 8
NODES_PER_CORE = N_NODES // N_CORES   # 1250
N_TILES = 10                          # 128-node tiles per core (1280 >= 1250)
CHUNK_BLKS = 8                        # blocks per decay/store chunk

# angular groups for MAX_L=3: sizes (l+1)(l+2)/2 = 1,3,6,10 -> starts 0,1,4,10
GROUP_SLOTS = [(0, 1), (1, 3), (4, 6), (10, 10)]
# PSUM layout: psA holds cols [0, ACT_COLS) (ACT-copied to bf16, then one
# big DVE 16-bit multiply per chunk); psC holds the tail (DVE multiplies it
# straight from PSUM in f32). Each matmul output stays inside a 2KB bank.
ACT_COLS = 960
MM_A = [(0, 512), (512, 448)]
MM_C = [(960, 320)]


def _chunks(total, step, first):
    chunks = [(0, min(first, total))]
    c = chunks[0][1]
    while c < total:
        w = min(step, total - c)
        chunks.append((c, w))
        c += w
    return chunks


def build_module(m_vec):
    """m_vec: blocks per node-tile (len N_TILES). Program is data-shaped
    only through m_vec, which the module cache keys on."""
    B = sum(m_vec)
    e_slots = B * 128
    tile_of_block = np.repeat(np.arange(N_TILES), m_vec)

    nc = bacc.Bacc(
        "TRN2",
        target_bir_lowering=False,
        debug=False,
        enable_asserts=False,
        num_devices=N_CORES,
    )
    table = nc.dram_tensor(
        "table", [N_TILES * 128, ROW], dt.bfloat16, kind="ExternalInput"
    ).ap()
    s_all = nc.dram_tensor(
        "s_all", [128, e_slots], dt.bfloat16, kind="ExternalInput"
    ).ap()
    rct = nc.dram_tensor("rct", [REMB, e_slots], dt.float32, kind="ExternalInput").ap()
    cutf = nc.dram_tensor("cutf", [128, B], dt.float32, kind="ExternalInput").ap()
    wt = nc.dram_tensor("wt", [REMB, GC], dt.float32, kind="ExternalInput").ap()
    msg = nc.dram_tensor("msg", [e_slots, ROW], dt.bfloat16, kind="ExternalOutput").ap()

    chunks = _chunks(B, CHUNK_BLKS, 4)
    n_act_r = ACT_COLS // (ANG * CH)
    n_dve_r = RADIAL - n_act_r

    with tile.TileContext(nc) as tc:
        with ExitStack() as ctx:
            const_pool = ctx.enter_context(tc.tile_pool(name="const", bufs=1))
            s_pool = ctx.enter_context(tc.tile_pool(name="s", bufs=4))
            rc_pool = ctx.enter_context(tc.tile_pool(name="rc", bufs=4))
            dec_pool = ctx.enter_context(tc.tile_pool(name="dec", bufs=4))
            cp_pool = ctx.enter_context(tc.tile_pool(name="cp", bufs=3))
            out_pool = ctx.enter_context(tc.tile_pool(name="out", bufs=3))
            psd_pool = ctx.enter_context(tc.tile_pool(name="psd", bufs=2, space="PSUM"))
            psa_pool = ctx.enter_context(tc.tile_pool(name="psa", bufs=2, space="PSUM"))
            psc_pool = ctx.enter_context(tc.tile_pool(name="psc", bufs=2, space="PSUM"))

            wt_sb = const_pool.tile([REMB, GC], dt.float32)
            nc.scalar.dma_start(wt_sb[:], wt[:, :])
            cut_sb = const_pool.tile([128, B], dt.float32)
            nc.scalar.dma_start(cut_sb[:], cutf[:, :])
            tab = []
            for t in range(N_TILES):
                tt = const_pool.tile([128, ROW], dt.bfloat16)
                nc.scalar.dma_start(tt[:], table[t * 128 : (t + 1) * 128, :])
                tab.append(tt)

            # block b's 128 output rows are DRAM rows [b*128, b*128+128)
            msg_v = msg.rearrange("(b p) e -> p b e", p=128)  # [128, B, ROW]

            # ---- decay prologue: all tiny rc@W matmuls run as one
            # uninterrupted PE burst (ramps the clock before the replication
            # stream), sigmoids land in a persistent dec32_all buffer, and
            # the steady-state loop keeps PE/ACT streams pure.
            dec32_all = const_pool.tile([128, B, GC], dt.float32)
            for c0, w in chunks:
                n_sl = w * 128
                rc_sb = rc_pool.tile([REMB, CHUNK_BLKS * 128], dt.float32, tag="rc")
                nc.sync.dma_start(
                    rc_sb[:, :n_sl], rct[:, c0 * 128 : c0 * 128 + n_sl]
                )
                psd = psd_pool.tile([128, CHUNK_BLKS * GC], dt.float32, tag="psd")
                for j in range(w):
                    nc.tensor.matmul(
                        out=psd[:, j * GC : (j + 1) * GC],
                        lhsT=rc_sb[:, j * 128 : (j + 1) * 128],
                        rhs=wt_sb[:],
                        start=True,
                        stop=True,
                    )
                nc.scalar.activation(
                    out=dec32_all[:, c0 : c0 + w, :],
                    in_=psd[:, : w * GC],
                    func=mybir.ActivationFunctionType.Sigmoid,
                )

            def decay_stage(c0, w):
                """S load + decay broadcast for a chunk (runs a chunk ahead)."""
                n_sl = w * 128
                s_sb = s_pool.tile([128, CHUNK_BLKS * 128], dt.bfloat16, tag="s")
                nc.sync.dma_start(
                    s_sb[:, :n_sl], s_all[:, c0 * 128 : c0 * 128 + n_sl]
                )
                # deca_bf[p, j, a, c] = dec32[p, j, g(a), c] * cutoff[p, j]
                deca_bf = dec_pool.tile(
                    [128, CHUNK_BLKS, ANG * CH], dt.bfloat16, tag="decabf"
                )
                cut_b = cut_sb[:, c0 : c0 + w]
                for g, (s0, ns) in enumerate(GROUP_SLOTS):
                    nc.gpsimd.tensor_mul(
                        out=deca_bf[:, :w, s0 * CH : (s0 + ns) * CH].rearrange(
                            "p w (n c) -> p w n c", c=CH
                        ),
                        in0=dec32_all[:, c0 : c0 + w, g * CH : (g + 1) * CH]
                        .unsqueeze(2)
                        .to_broadcast([128, w, ns, CH]),
                        in1=cut_b.unsqueeze(2)
                        .unsqueeze(3)
                        .to_broadcast([128, w, ns, CH]),
                    )
                return s_sb, deca_bf

            def multiply_stage(c0, w, state):
                s_sb, deca_bf = state
                out_t = out_pool.tile([128, CHUNK_BLKS, ROW], dt.bfloat16, tag="out")
                cpc = cp_pool.tile([128, CHUNK_BLKS, ACT_COLS], dt.bfloat16, tag="cp")
                for j in range(w):
                    b = c0 + j
                    t = int(tile_of_block[b])
                    psa = psa_pool.tile([128, ACT_COLS], dt.float32, tag="psa")
                    psc = psc_pool.tile([128, ROW - ACT_COLS], dt.float32, tag="psc")
                    first = True
                    for ps_t, splits in ((psa, MM_A), (psc, MM_C)):
                        for f0, nf in splits:
                            o0 = 0 if ps_t is psc else f0
                            inst = nc.tensor.matmul(
                                out=ps_t[:, o0 : o0 + nf],
                                lhsT=s_sb[:, j * 128 : (j + 1) * 128],
                                rhs=tab[t][:, f0 : f0 + nf],
                                start=True,
                                stop=True,
                            )
                            # the same S stays resident in the PE array:
                            # only the first matmul of the block loads it
                            if not first:
                                inst.ldweights = False
                            first = False
                    # ACT evacuates cols [0, ACT_COLS) to bf16 ...
                    nc.scalar.activation(
                        out=cpc[:, j, :],
                        in_=psa[:, :],
                        func=mybir.ActivationFunctionType.Copy,
                    )
                    # ... and DVE handles the tail straight from PSUM (f32).
                    nc.vector.tensor_mul(
                        out=out_t[:, j, ACT_COLS:].rearrange(
                            "p (r ac) -> p r ac", ac=ANG * CH
                        ),
                        in0=psc[:, :].rearrange(
                            "p (r ac) -> p r ac", ac=ANG * CH
                        ),
                        in1=deca_bf[:, j, :]
                        .unsqueeze(1)
                        .to_broadcast([128, n_dve_r, ANG * CH]),
                    )
                # one big 16-bit DVE multiply over the whole chunk's copies
                nc.vector.tensor_mul(
                    out=out_t[:, :w, :ACT_COLS].rearrange(
                        "p w (r ac) -> p w r ac", ac=ANG * CH
                    ),
                    in0=cpc[:, :w, :].rearrange("p w (r ac) -> p w r ac", ac=ANG * CH),
                    in1=deca_bf[:, :w, :]
                    .unsqueeze(2)
                    .to_broadcast([128, w, n_act_r, ANG * CH]),
                )
                for p0 in range(0, w, 2):
                    pw = min(2, w - p0)
                    nc.sync.dma_start(
                        out=msg_v[:, c0 + p0 : c0 + p0 + pw, :],
                        in_=out_t[:, p0 : p0 + pw, :],
                    )

            # software pipeline: decay for chunk i+1 is queued before the
            # multiplies of chunk i, so ACT's sigmoid and Pool's broadcasts
            # never sit behind the copy stream in their FIFOs.
            state = decay_stage(*chunks[0])
            for ci, (c0, w) in enumerate(chunks):
                nxt = decay_stage(*chunks[ci + 1]) if ci + 1 < len(chunks) else None
                multiply_stage(c0, w, state)
                state = nxt

    nc.compile()
    return nc


def _f32_to_bf16(a):
    """Round-to-nearest-even f32 -> bf16, as raw uint16-backed bf16 array."""
    u = np.ascontiguousarray(a, dtype=np.float32).view(np.uint32)
    r = ((u + 0x7FFF + ((u >> 16) & 1)) >> 16).astype(np.uint16)
    return r.view(ml_dtypes.bfloat16)


def prep(node_feat, radial_component, radial_cutoff_fn, weights, edge_index):
    """Host-side sort/shard/layout. Returns (m_vec, in_maps, aux) where aux
    holds per-core (orig_edge_ids, slots) for unpermuting the output."""
    node_bf = _f32_to_bf16(np.asarray(node_feat).reshape(N_NODES, ROW))
    wt = np.ascontiguousarray(
        np.asarray(weights, dtype=np.float32).transpose(1, 0, 2).reshape(REMB, GC)
    )
    senders = np.asarray(edge_index)[0].astype(np.int64)
    rc_all = np.asarray(radial_component, dtype=np.float32)
    cut_all = np.asarray(radial_cutoff_fn, dtype=np.float32)

    order = np.argsort(senders, kind="stable")
    ss = senders[order]
    bounds = np.searchsorted(ss, np.arange(N_CORES + 1) * NODES_PER_CORE)

    cores = []
    m_vec = np.ones(N_TILES, np.int64)
    for i in range(N_CORES):
        lo, hi = int(bounds[i]), int(bounds[i + 1])
        ids = order[lo:hi]
        off = ss[lo:hi] - NODES_PER_CORE * i
        tile_id = off >> 7
        row_rel = off & 127
        counts = np.bincount(tile_id, minlength=N_TILES)
        m_vec = np.maximum(m_vec, -(-counts // 128))
        starts = np.concatenate([[0], np.cumsum(counts)[:-1]])
        pos_in_tile = np.arange(hi - lo) - starts[tile_id]
        cores.append((ids, tile_id, row_rel, pos_in_tile))

    m_vec = tuple(int(v) for v in m_vec)
    B = sum(m_vec)
    e_slots = B * 128
    blk_start = np.concatenate([[0], np.cumsum(m_vec)])  # first block of tile t

    in_maps, aux = [], []
    for i in range(N_CORES):
        ids, tile_id, row_rel, pos_in_tile = cores[i]
        slot = (blk_start[tile_id] * 128 + pos_in_tile).astype(np.int64)

        s_dram = np.zeros((128, e_slots), ml_dtypes.bfloat16)
        s_dram[row_rel, slot] = 1.0
        rc_dram = np.zeros((REMB, e_slots), np.float32)
        rc_dram[:, slot] = rc_all[ids].T
        cut_dram = np.zeros((128, B), np.float32)
        cut_dram[slot & 127, slot >> 7] = cut_all[ids]

        n0 = NODES_PER_CORE * i
        n1 = min(n0 + N_TILES * 128, N_NODES)
        tab = np.zeros((N_TILES * 128, ROW), ml_dtypes.bfloat16)
        tab[: n1 - n0] = node_bf[n0:n1]

        in_maps.append(
            {"table": tab, "s_all": s_dram, "rct": rc_dram,
             "cutf": cut_dram, "wt": wt}
        )
        aux.append((ids, slot))
    return m_vec, in_maps, aux


_nc_cache = {}


def get_module(m_vec):
    if m_vec not in _nc_cache:
        _nc_cache[m_vec] = build_module(m_vec)
    return _nc_cache[m_vec]


def unshard(results, aux):
    out = np.empty((E_TOTAL, ROW), np.float32)
    for (ids, slot), r in zip(aux, results):
        u = r["msg"].view(np.uint16)[slot].astype(np.uint32) << 16
        out[ids] = u.view(np.float32)
    return out.reshape(E_TOTAL, RADIAL, ANG, CH)


def kernel(node_feat, radial_component, radial_cutoff_fn, weights, edge_index):
    m_vec, in_maps, aux = prep(
        node_feat, radial_component, radial_cutoff_fn, weights, edge_index
    )
    nc = get_module(m_vec)
    res = run_bass_kernel_spmd(nc, in_maps, core_ids=list(range(N_CORES)))
    return unshard(res.results, aux)


# revision 28
# speedup vs baseline: 1.0374x; 1.0374x over previous
"""Trainium2 Bass kernel for nn_MessageArMLP (GNN message passing).

message[e, r, a, c] = node_feat[sender[e], r, a, c]
                      * sigmoid(rc[e] @ W[group(a)])[c] * cutoff[e]

Strategy (PE replication, no per-edge DMA gather):
  * Host sorts edges by sender; core i owns edges whose sender is in
    nodes [1250*i, 1250*(i+1)).  Senders within a core are grouped into
    ten 128-node tiles; each tile's edges fill m_t 128-edge blocks.
  * The core's 1280-node slice of node_feat (bf16) is DMA'd contiguously
    into SBUF once — each node row crosses HBM exactly once (vs. 12x for
    a per-edge gather).
  * Per 128-edge block, a 128x128 0/1 selection matrix S (built on host)
    replicates sender rows via a PE matmul chain (512/448 cols into psA,
    320 into psC — each inside one 2KB PSUM bank):
        psum[e, :] = sum_n S[n, e] * tile[n, :]
  * PSUM evacuation is split across engines: ACT copies cols [0,960) to
    SBUF bf16 and one big DVE 16-bit multiply per chunk applies the
    decay at ~2x rate; DVE multiplies cols [960,1280) straight out of
    PSUM (f32 x bf16 decay).  Pool builds the per-edge decay broadcast
    (sigmoid(rc@W[g]) x cutoff) a chunk ahead of use.  Output is bf16
    (the 2e-2 rel-err budget dwarfs bf16's ~1% worst-case error); the
    host upcasts to f32 and unpermutes to original edge order.
  * HBM traffic/core: ~41.6 MB store + ~8.5 MB loads, no gather reads.
  * Measured: ~246 us vs 496 us for the f32 DMA-gather baseline.
"""

import numpy as np
import ml_dtypes
from contextlib import ExitStack

import concourse.tile as tile
from concourse import bacc, mybir
from concourse.bass_utils import run_bass_kernel_spmd

dt = mybir.dt

# Problem constants (hardcoded per harness contract)
N_NODES = 10000
E_TOTAL = 120000
RADIAL = 8
ANG = 20
CH = 8
REMB = 8
ROW = RADIAL * ANG * CH     # 1280 elems per node row (2560 B bf16)
G = 4
GC = G * CH                 # 32
N_CORES = 8
NODES_PER_CORE = N_NODES // N_CORES   # 1250
N_TILES = 10                          # 128-node tiles per core (1280 >= 1250)
CHUNK_BLKS = 8                        # blocks per decay/store chunk

# angular groups for MAX_L=3: sizes (l+1)(l+2)/2 = 1,3,6,10 -> starts 0,1,4,10
GROUP_SLOTS = [(0, 1), (1, 3), (4, 6), (10, 10)]
# PSUM layout: psA holds cols [0, ACT_COLS) (ACT-copied to bf16, then one
# big DVE 16-bit multiply per chunk); psC holds the tail (DVE multiplies it
# straight from PSUM in f32). Each matmul output stays inside a 2KB bank.
ACT_COLS = 960
MM_A = [(0, 512), (512, 448)]
MM_C = [(960, 320)]


def _chunks(total, step, first):
    chunks = [(0, min(first, total))]
    c = chunks[0][1]
    while c < total:
        w = min(step, total - c)
        chunks.append((c, w))
        c += w
    return chunks


def build_module(m_vec):
    """m_vec: blocks per node-tile (len N_TILES). Program is data-shaped
    only through m_vec, which the module cache keys on."""
    B = sum(m_vec)
    e_slots = B * 128
    tile_of_block = np.repeat(np.arange(N_TILES), m_vec)

    nc = bacc.Bacc(
        "TRN2",
        target_bir_lowering=False,
        debug=False,
        enable_asserts=False,
        num_devices=N_CORES,
    )
    table = nc.dram_tensor(
        "table", [N_TILES * 128, ROW], dt.bfloat16, kind="ExternalInput"
    ).ap()
    s_all = nc.dram_tensor(
        "s_all", [128, e_slots], dt.bfloat16, kind="ExternalInput"
    ).ap()
    rct = nc.dram_tensor("rct", [REMB, e_slots], dt.float32, kind="ExternalInput").ap()
    cutf = nc.dram_tensor("cutf", [128, B], dt.float32, kind="ExternalInput").ap()
    wt = nc.dram_tensor("wt", [REMB, GC], dt.float32, kind="ExternalInput").ap()
    msg = nc.dram_tensor("msg", [e_slots, ROW], dt.bfloat16, kind="ExternalOutput").ap()

    chunks = _chunks(B, CHUNK_BLKS, 4)
    n_act_r = ACT_COLS // (ANG * CH)
    n_dve_r = RADIAL - n_act_r

    with tile.TileContext(nc) as tc:
        with ExitStack() as ctx:
            const_pool = ctx.enter_context(tc.tile_pool(name="const", bufs=1))
            s_pool = ctx.enter_context(tc.tile_pool(name="s", bufs=6))
            rc_pool = ctx.enter_context(tc.tile_pool(name="rc", bufs=4))
            dec_pool = ctx.enter_context(tc.tile_pool(name="dec", bufs=4))
            cp_pool = ctx.enter_context(tc.tile_pool(name="cp", bufs=4))
            out_pool = ctx.enter_context(tc.tile_pool(name="out", bufs=4))
            psd_pool = ctx.enter_context(tc.tile_pool(name="psd", bufs=2, space="PSUM"))
            psa_pool = ctx.enter_context(tc.tile_pool(name="psa", bufs=2, space="PSUM"))
            psc_pool = ctx.enter_context(tc.tile_pool(name="psc", bufs=2, space="PSUM"))

            wt_sb = const_pool.tile([REMB, GC], dt.float32)
            nc.scalar.dma_start(wt_sb[:], wt[:, :])
            cut_sb = const_pool.tile([128, B], dt.float32)
            nc.scalar.dma_start(cut_sb[:], cutf[:, :])
            tab = []
            for t in range(N_TILES):
                tt = const_pool.tile([128, ROW], dt.bfloat16)
                nc.scalar.dma_start(tt[:], table[t * 128 : (t + 1) * 128, :])
                tab.append(tt)

            # block b's 128 output rows are DRAM rows [b*128, b*128+128)
            msg_v = msg.rearrange("(b p) e -> p b e", p=128)  # [128, B, ROW]

            def decay_stage(c0, w):
                """Loads + decay pipeline for a chunk (runs a chunk ahead)."""
                n_sl = w * 128
                s_sb = s_pool.tile([128, CHUNK_BLKS * 128], dt.bfloat16, tag="s")
                nc.sync.dma_start(
                    s_sb[:, :n_sl], s_all[:, c0 * 128 : c0 * 128 + n_sl]
                )
                rc_sb = rc_pool.tile([REMB, CHUNK_BLKS * 128], dt.float32, tag="rc")
                nc.sync.dma_start(
                    rc_sb[:, :n_sl], rct[:, c0 * 128 : c0 * 128 + n_sl]
                )
                psd = psd_pool.tile([128, CHUNK_BLKS * GC], dt.float32, tag="psd")
                for j in range(w):
                    nc.tensor.matmul(
                        out=psd[:, j * GC : (j + 1) * GC],
                        lhsT=rc_sb[:, j * 128 : (j + 1) * 128],
                        rhs=wt_sb[:],
                        start=True,
                        stop=True,
                    )
                dec32 = dec_pool.tile([128, CHUNK_BLKS, GC], dt.float32, tag="dec32")
                nc.scalar.activation(
                    out=dec32[:, :w, :],
                    in_=psd[:, : w * GC],
                    func=mybir.ActivationFunctionType.Sigmoid,
                )
                # deca_bf[p, j, a, c] = dec32[p, j, g(a), c] * cutoff[p, j]
                deca_bf = dec_pool.tile(
                    [128, CHUNK_BLKS, ANG * CH], dt.bfloat16, tag="decabf"
                )
                cut_b = cut_sb[:, c0 : c0 + w]
                for g, (s0, ns) in enumerate(GROUP_SLOTS):
                    nc.gpsimd.tensor_mul(
                        out=deca_bf[:, :w, s0 * CH : (s0 + ns) * CH].rearrange(
                            "p w (n c) -> p w n c", c=CH
                        ),
                        in0=dec32[:, :w, g * CH : (g + 1) * CH]
                        .unsqueeze(2)
                        .to_broadcast([128, w, ns, CH]),
                        in1=cut_b.unsqueeze(2)
                        .unsqueeze(3)
                        .to_broadcast([128, w, ns, CH]),
                    )
                return s_sb, deca_bf

            def multiply_stage(c0, w, state):
                s_sb, deca_bf = state
                out_t = out_pool.tile([128, CHUNK_BLKS, ROW], dt.bfloat16, tag="out")
                cpc = cp_pool.tile([128, CHUNK_BLKS, ACT_COLS], dt.bfloat16, tag="cp")
                for j in range(w):
                    b = c0 + j
                    t = int(tile_of_block[b])
                    psa = psa_pool.tile([128, ACT_COLS], dt.float32, tag="psa")
                    psc = psc_pool.tile([128, ROW - ACT_COLS], dt.float32, tag="psc")
                    first = True
                    for ps_t, splits in ((psa, MM_A), (psc, MM_C)):
                        for f0, nf in splits:
                            o0 = 0 if ps_t is psc else f0
                            inst = nc.tensor.matmul(
                                out=ps_t[:, o0 : o0 + nf],
                                lhsT=s_sb[:, j * 128 : (j + 1) * 128],
                                rhs=tab[t][:, f0 : f0 + nf],
                                start=True,
                                stop=True,
                            )
                            # the same S stays resident in the PE array:
                            # only the first matmul of the block loads it
                            if not first:
                                inst.ldweights = False
                            first = False
                    # ACT evacuates cols [0, ACT_COLS) to bf16 ...
                    nc.scalar.activation(
                        out=cpc[:, j, :],
                        in_=psa[:, :],
                        func=mybir.ActivationFunctionType.Copy,
                    )
                    # ... and DVE handles the tail straight from PSUM (f32).
                    nc.vector.tensor_mul(
                        out=out_t[:, j, ACT_COLS:].rearrange(
                            "p (r ac) -> p r ac", ac=ANG * CH
                        ),
                        in0=psc[:, :].rearrange(
                            "p (r ac) -> p r ac", ac=ANG * CH
                        ),
                        in1=deca_bf[:, j, :]
                        .unsqueeze(1)
                        .to_broadcast([128, n_dve_r, ANG * CH]),
                    )
                # one big 16-bit DVE multiply over the whole chunk's copies
                nc.vector.tensor_mul(
                    out=out_t[:, :w, :ACT_COLS].rearrange(
                        "p w (r ac) -> p w r ac", ac=ANG * CH
                    ),
                    in0=cpc[:, :w, :].rearrange("p w (r ac) -> p w r ac", ac=ANG * CH),
                    in1=deca_bf[:, :w, :]
                    .unsqueeze(2)
                    .to_broadcast([128, w, n_act_r, ANG * CH]),
                )
                for p0 in range(0, w, 2):
                    pw = min(2, w - p0)
                    nc.sync.dma_start(
                        out=msg_v[:, c0 + p0 : c0 + p0 + pw, :],
                        in_=out_t[:, p0 : p0 + pw, :],
                    )

            # software pipeline: decay for chunk i+1 is queued before the
            # multiplies of chunk i, so ACT's sigmoid and Pool's broadcasts
            # never sit behind the copy stream in their FIFOs.
            state = decay_stage(*chunks[0])
            for ci, (c0, w) in enumerate(chunks):
                nxt = decay_stage(*chunks[ci + 1]) if ci + 1 < len(chunks) else None
                multiply_stage(c0, w, state)
                state = nxt

    nc.compile()
    return nc


def _f32_to_bf16(a):
    """Round-to-nearest-even f32 -> bf16, as raw uint16-backed bf16 array."""
    u = np.ascontiguousarray(a, dtype=np.float32).view(np.uint32)
    r = ((u + 0x7FFF + ((u >> 16) & 1)) >> 16).astype(np.uint16)
    return r.view(ml_dtypes.bfloat16)


def prep(node_feat, radial_component, radial_cutoff_fn, weights, edge_index):
    """Host-side sort/shard/layout. Returns (m_vec, in_maps, aux) where aux
    holds per-core (orig_edge_ids, slots) for unpermuting the output."""
    node_bf = _f32_to_bf16(np.asarray(node_feat).reshape(N_NODES, ROW))
    wt = np.ascontiguousarray(
        np.asarray(weights, dtype=np.float32).transpose(1, 0, 2).reshape(REMB, GC)
    )
    senders = np.asarray(edge_index)[0].astype(np.int64)
    rc_all = np.asarray(radial_component, dtype=np.float32)
    cut_all = np.asarray(radial_cutoff_fn, dtype=np.float32)

    order = np.argsort(senders, kind="stable")
    ss = senders[order]
    bounds = np.searchsorted(ss, np.arange(N_CORES + 1) * NODES_PER_CORE)

    cores = []
    m_vec = np.ones(N_TILES, np.int64)
    for i in range(N_CORES):
        lo, hi = int(bounds[i]), int(bounds[i + 1])
        ids = order[lo:hi]
        off = ss[lo:hi] - NODES_PER_CORE * i
        tile_id = off >> 7
        row_rel = off & 127
        counts = np.bincount(tile_id, minlength=N_TILES)
        m_vec = np.maximum(m_vec, -(-counts // 128))
        starts = np.concatenate([[0], np.cumsum(counts)[:-1]])
        pos_in_tile = np.arange(hi - lo) - starts[tile_id]
        cores.append((ids, tile_id, row_rel, pos_in_tile))

    m_vec = tuple(int(v) for v in m_vec)
    B = sum(m_vec)
    e_slots = B * 128
    blk_start = np.concatenate([[0], np.cumsum(m_vec)])  # first block of tile t

    in_maps, aux = [], []
    for i in range(N_CORES):
        ids, tile_id, row_rel, pos_in_tile = cores[i]
        slot = (blk_start[tile_id] * 128 + pos_in_tile).astype(np.int64)

        s_dram = np.zeros((128, e_slots), ml_dtypes.bfloat16)
        s_dram[row_rel, slot] = 1.0
        rc_dram = np.zeros((REMB, e_slots), np.float32)
        rc_dram[:, slot] = rc_all[ids].T
        cut_dram = np.zeros((128, B), np.float32)
        cut_dram[slot & 127, slot >> 7] = cut_all[ids]

        n0 = NODES_PER_CORE * i
        n1 = min(n0 + N_TILES * 128, N_NODES)
        tab = np.zeros((N_TILES * 128, ROW), ml_dtypes.bfloat16)
        tab[: n1 - n0] = node_bf[n0:n1]

        in_maps.append(
            {"table": tab, "s_all": s_dram, "rct": rc_dram,
             "cutf": cut_dram, "wt": wt}
        )
        aux.append((ids, slot))
    return m_vec, in_maps, aux


_nc_cache = {}


def get_module(m_vec):
    if m_vec not in _nc_cache:
        _nc_cache[m_vec] = build_module(m_vec)
    return _nc_cache[m_vec]


def unshard(results, aux):
    out = np.empty((E_TOTAL, ROW), np.float32)
    for (ids, slot), r in zip(aux, results):
        u = r["msg"].view(np.uint16)[slot].astype(np.uint32) << 16
        out[ids] = u.view(np.float32)
    return out.reshape(E_TOTAL, RADIAL, ANG, CH)


def kernel(node_feat, radial_component, radial_cutoff_fn, weights, edge_index):
    m_vec, in_maps, aux = prep(
        node_feat, radial_component, radial_cutoff_fn, weights, edge_index
    )
    nc = get_module(m_vec)
    res = run_bass_kernel_spmd(nc, in_maps, core_ids=list(range(N_CORES)))
    return unshard(res.results, aux)
